# revision 7
# baseline (speedup 1.0000x reference)
"""DND estimator (retrieval kNN) Trainium2 kernel.

Pipeline (SPMD over 8 NeuronCores, DND table sharded along N):
  - encoder computed on device in transposed orientation:
    hT = relu(W1^T-contract) with per-partition bias; policy head from 2*hT
  - per core: scores s = 2*h @ K_shard^T - ||k||^2 via one augmented matmul
    (65-dim contraction: [2h; 1] x [K^T; -ksq]); per-row ranking by s is the
    same as by squared distance d = ||h||^2 - s  (||h||^2 is a row constant)
  - per 2048-col segment of the [128, Nshard] score tile (PSUM -> SBUF), the
    DVE max8/max_index ops extract the top-8 values + indices; the union over
    segments covers each row's global top-50 w.p. ~1-4e-4 (Poisson bound)
  - candidates (values f32 + u32 local indices) DMA'd out; the host does the
    tiny 8-way merge: exact top-50 of ~832 candidates/row, inverse-distance
    weights, and the vals_mem gather.
"""

import os
import sys

import numpy as np

for _p in ("/opt/trn_rl_repo", "/root/.axon_site/_ro/trn_rl_repo"):
    if os.path.isdir(_p) and _p not in sys.path:
        sys.path.insert(0, _p)

B = 1024
S = 256
H = 64
A = 18
N = 200000
NCORES = 8
NSH = N // NCORES          # 25000 keys per core
CH = 512                   # psum chunk (one f32 bank)
NCHUNK = 49                # ceil(25000/512)
NSHP = NCHUNK * CH         # 25088 padded shard width
SEG_CH = 4                 # chunks per max8 segment
NSEG = 13                  # 12 segs of 2048 + 1 seg of 512
CANDW = NSEG * 8           # 104 candidates per row per core
KNN = 50
DELTA = 1e-3

_CACHE = {}


def _build_bass():
    import concourse.tile as tile
    from concourse import bacc, mybir
    from contextlib import ExitStack

    f32 = mybir.dt.float32
    u32 = mybir.dt.uint32

    nc = bacc.Bacc("TRN2", target_bir_lowering=False, debug=False,
                   num_devices=NCORES)

    xT = nc.dram_tensor("xT", [S, B], f32, kind="ExternalInput").ap()
    W1d = nc.dram_tensor("W1d", [S, H], f32, kind="ExternalInput").ap()
    b1c = nc.dram_tensor("b1c", [H, 1], f32, kind="ExternalInput").ap()
    Wph = nc.dram_tensor("Wph", [H, A], f32, kind="ExternalInput").ap()
    KTaug = nc.dram_tensor("KTaug", [H + 1, NSHP], f32, kind="ExternalInput").ap()

    hT_out = nc.dram_tensor("hT_out", [H, B], f32, kind="ExternalOutput").ap()
    pol_out = nc.dram_tensor("policy_out", [B, A], f32, kind="ExternalOutput").ap()
    cv_out = nc.dram_tensor("cand_vals", [B, CANDW], f32, kind="ExternalOutput").ap()
    ci_out = nc.dram_tensor("cand_idx", [B, CANDW], u32, kind="ExternalOutput").ap()

    AF = mybir.ActivationFunctionType

    with tile.TileContext(nc) as tc, ExitStack() as ctx:
        const = ctx.enter_context(tc.tile_pool(name="const", bufs=1))

        xt0 = const.tile([128, B], f32, tag="xt0")
        xt1 = const.tile([128, B], f32, tag="xt1")
        nc.sync.dma_start(xt0[:], xT[0:128, :])
        nc.sync.dma_start(xt1[:], xT[128:256, :])

        w10 = const.tile([128, H], f32, tag="w10")
        w11 = const.tile([128, H], f32, tag="w11")
        nc.sync.dma_start(w10[:], W1d[0:128, :])
        nc.sync.dma_start(w11[:], W1d[128:256, :])

        b1sb = const.tile([H, 1], f32, tag="b1sb")
        nc.sync.dma_start(b1sb[:], b1c[:, :])
        wph = const.tile([H, A], f32, tag="wph")
        nc.sync.dma_start(wph[:], Wph[:, :])

        # full key shard resident in SBUF (~6.5 MB)
        kt = const.tile([H + 1, NSHP], f32, tag="kt")
        NDMA = 7
        per = NCHUNK // NDMA
        for i in range(NDMA):
            lo = i * per * CH
            hi = NSHP if i == NDMA - 1 else (i + 1) * per * CH
            nc.sync.dma_start(kt[:, lo:hi], KTaug[:, lo:hi])

        # [2*h^T ; ones] stationary operand, all of B
        haugt = const.tile([H + 1, B], f32, tag="haugt")
        nc.vector.memset(haugt[H:H + 1, :], 1.0)

        # ---- phase 1: encoder (transposed orientation) + policy ----
        with tc.tile_pool(name="ph1_psum", bufs=2, space="PSUM") as pp1, \
             tc.tile_pool(name="ph1_sb", bufs=2) as sb1:
            for hc in range(2):  # B in two 512-wide halves (f32 moving max)
                cs = slice(hc * 512, (hc + 1) * 512)
                ph = pp1.tile([H, 512], f32, tag="ph")
                nc.tensor.matmul(ph[:], w10[:], xt0[:, cs], start=True, stop=False)
                nc.tensor.matmul(ph[:], w11[:], xt1[:, cs], start=False, stop=True)
                # hT (exact, for the h output)
                hsb = sb1.tile([H, 512], f32, tag="hsb")
                nc.scalar.activation(hsb[:], ph[:], AF.Relu, bias=b1sb[:, 0:1])
                nc.sync.dma_start(hT_out[:, cs], hsb[:])
                # 2*hT rows of the stationary operand: relu(2x+2b) = 2relu(x+b)
                nc.scalar.activation(haugt[0:H, cs], ph[:], AF.Relu, scale=2.0,
                                     bias=b1sb[:, 0:1])
            for bt in range(8):
                bs = slice(bt * 128, (bt + 1) * 128)
                ppol = pp1.tile([128, A], f32, tag="ppol")
                nc.tensor.matmul(ppol[:], haugt[0:H, bs], wph[:],
                                 start=True, stop=True)
                psb = sb1.tile([128, A], f32, tag="psb")
                nc.scalar.activation(psb[:], ppol[:], AF.Copy)
                nc.sync.dma_start(pol_out[bs, :], psb[:])

        # ---- phase 2: scores + top-8 per segment ----
        _nbt = int(os.environ.get("KB_BT", "8"))
        _nseg = int(os.environ.get("KB_SEG", str(NSEG)))
        with tc.tile_pool(name="spsum", bufs=6, space="PSUM") as pps, \
             tc.tile_pool(name="seg", bufs=3) as segp, \
             tc.tile_pool(name="cand", bufs=2) as candp:
            for bt in range(_nbt):
                bs = slice(bt * 128, (bt + 1) * 128)
                cv = candp.tile([128, CANDW], f32, tag="cv")
                ci = candp.tile([128, CANDW], u32, tag="ci")
                for seg in range(_nseg):
                    nch = SEG_CH if seg < NSEG - 1 else NCHUNK - SEG_CH * (NSEG - 1)
                    w = nch * CH
                    ssb = segp.tile([128, SEG_CH * CH], f32, tag="ssb")
                    for j in range(nch):
                        ch = seg * SEG_CH + j
                        ps = pps.tile([128, CH], f32, tag="ps")
                        nc.tensor.matmul(ps[:], haugt[:, bs],
                                         kt[:, ch * CH:(ch + 1) * CH],
                                         start=True, stop=True)
                        nc.scalar.activation(ssb[:, j * CH:(j + 1) * CH], ps[:],
                                             AF.Copy)
                    co = slice(seg * 8, seg * 8 + 8)
                    nc.vector.max(cv[:, co], ssb[:, 0:w])
                    nc.vector.max_index(ci[:, co], cv[:, co], ssb[:, 0:w])
                nc.sync.dma_start(cv_out[bs, :], cv[:])
                nc.sync.dma_start(ci_out[bs, :], ci[:])

    nc.compile()
    return nc


def _get_nc():
    if "nc" not in _CACHE:
        _CACHE["nc"] = _build_bass()
    return _CACHE["nc"]


def _prep_inputs(x, W1, b1, Wp, bp, keys_mem, vals_mem):
    """Host-side input reformatting (sharding + transposes + norm row)."""
    xTh = np.ascontiguousarray(x.T).astype(np.float32)
    W1d = np.ascontiguousarray(W1).astype(np.float32)
    # device applies relu(scale*x + bias) with ONE bias column for both the
    # exact-h pass (scale=1) and the doubled pass (scale=2).  b1 is zeros in
    # this problem so one column serves both; assert to be safe.
    assert np.abs(b1).max() == 0.0, "nonzero b1 needs two bias columns"
    b1c = np.zeros((H, 1), np.float32)
    Wph = (0.5 * Wp).astype(np.float32)

    ksq = np.einsum("nh,nh->n", keys_mem.astype(np.float64),
                    keys_mem.astype(np.float64))
    in_maps = []
    for c in range(NCORES):
        sl = slice(c * NSH, (c + 1) * NSH)
        KTaug = np.empty((H + 1, NSHP), np.float32)
        KTaug[:H, :NSH] = keys_mem[sl].T
        KTaug[:H, NSH:] = 0.0
        KTaug[H, :NSH] = (-ksq[sl]).astype(np.float32)
        KTaug[H, NSH:] = -1e30  # pad columns: never selected
        in_maps.append({
            "xT": xTh, "W1d": W1d, "b1c": b1c, "Wph": Wph, "KTaug": KTaug,
        })
    return in_maps


def _install_ntff_hook():
    """The agent image's antenv lacks axon_hooks; shim it so trace=True can
    capture NTFF profiles via the injected libaxon_pjrt.so ctypes path."""
    import types
    try:
        from antenv.axon_hooks import get_axon_ntff_profile_hook  # noqa: F401
        return
    except ImportError:
        pass
    try:
        from trn_agent_boot.trn_boot import _ntff_profile_via_ctypes
        hook = _ntff_profile_via_ctypes("/opt/axon/libaxon_pjrt.so")
    except Exception:
        hook = None
    mod = types.ModuleType("antenv.axon_hooks")
    mod.get_axon_ntff_profile_hook = lambda: hook
    mod.set_axon_ntff_profile_hook = lambda h: None
    import antenv
    sys.modules["antenv.axon_hooks"] = mod
    antenv.axon_hooks = mod


def _run_device(in_maps, trace=False):
    from concourse import bass_utils
    if trace:
        _install_ntff_hook()
    nc = _get_nc()
    res = bass_utils.run_bass_kernel_spmd(
        nc, in_maps, core_ids=list(range(NCORES)), trace=trace)
    return res.results, res.exec_time_ns


def _merge(results, bp, vals_mem):
    """Host merge: global top-50 across per-core candidates + weighting."""
    h = np.ascontiguousarray(results[0]["hT_out"].T)
    policy = results[0]["policy_out"] + bp[None, :].astype(np.float32)

    svals = np.concatenate([results[c]["cand_vals"] for c in range(NCORES)],
                           axis=1)                       # [B, 8*CANDW]
    lidx = np.concatenate([results[c]["cand_idx"] for c in range(NCORES)],
                          axis=1).astype(np.int64)       # local idx in segment
    ncand = CANDW * NCORES
    cols = np.arange(ncand)
    core = cols // CANDW
    seg = (cols % CANDW) // 8
    gbase = core * NSH + seg * (SEG_CH * CH)             # segment start, global
    gidx = lidx + gbase[None, :]

    # top-50 by score descending == by distance ascending
    part = np.argpartition(-svals, KNN - 1, axis=1)[:, :KNN]
    s_top = np.take_along_axis(svals, part, axis=1)
    g_top = np.take_along_axis(gidx, part, axis=1)

    hsq = np.einsum("bh,bh->b", h.astype(np.float64),
                    h.astype(np.float64)).astype(np.float32)
    d = hsq[:, None] - s_top
    dist = np.maximum(d, 0.0)
    wgt = 1.0 / (dist + DELTA)
    wgt = wgt / wgt.sum(axis=1, keepdims=True)
    v_nb = vals_mem[g_top, 0]
    value = (wgt * v_nb).sum(axis=1, keepdims=True).astype(np.float32)
    return policy.astype(np.float32), value, h.astype(np.float32)


def kernel(x, W1, b1, Wp, bp, keys_mem, vals_mem):
    in_maps = _prep_inputs(x, W1, b1, Wp, bp, keys_mem, vals_mem)
    results, _ = _run_device(in_maps, trace=False)
    return _merge(results, bp, vals_mem)
